# revision 4
# baseline (speedup 1.0000x reference)
"""KernelDensityEstimate Trainium kernel.

prob[n,m] = (sum_q exp(-0.5*invvar*||a_n - b_{m,q}||^2)) / (row_sum + 1e-10)

All exponents here are <= -94, so every density underflows f32; the reference's
nonzero outputs come from subnormal exp values divided by the 1e-10 epsilon.
We compute exp(t + S) with S=16.636 (so the surviving values are normal f32 and
the f32-exp flush threshold lands exactly where the reference's subnormal
flush-to-zero threshold is), then divide by 1e-10*e^S.

Device work (per core, data-parallel over N: 512 rows each):
  64 mq-tiles of 128 rows; per tile:
    MM (bf16, K=2)   psum  = ones (x) (c*a2)            [hi/lo split rows]
    MM (fp32r,K=128) psum += (-2c) * b_tile . a^T
    ACT Exp          dens  = exp(psum + (c*b2 + S))     -> bf16 SBUF
    MM (bf16, K=128) dpc  += blockones . dens           [Q-sum, accumulates]
  Tail: dpc psum -> SBUF f32, DMA out.
Host: normalization row-division (0.01% of FLOPs).
"""
import os
import sys
sys.path.insert(0, "/opt/trn_rl_repo")
import numpy as np
import ml_dtypes

N, M, Q, D = 4096, 128, 64, 128
NCORES = 8
NSH = N // NCORES          # 512 rows per core
NT = (M * Q) // 128        # 64 mq tiles
S_SHIFT = 16.636

_cache = {}


def _build(ps_bufs=6, dens_bufs=4):
    import concourse.bass as bass
    import concourse.mybir as mybir

    F32, F32R, BF16 = mybir.dt.float32, mybir.dt.float32r, mybir.dt.bfloat16
    AF = mybir.ActivationFunctionType

    nc = bass.Bass()
    d_mega = nc.declare_dram_parameter("mega", [128, 8192 + NSH], F32R, isOutput=False)
    d_bpk = nc.declare_dram_parameter("bpk", [128, 2 * 8192 + NSH], BF16, isOutput=False)
    d_dpc = nc.declare_dram_parameter("dpc", [128, NSH], F32, isOutput=True)

    PSB, DB = ps_bufs, dens_bufs
    with (
        nc.sbuf_tensor([128, 8192 + NSH], F32R) as mega,
        nc.sbuf_tensor([128, 2 * 8192 + NSH], BF16) as bpk,
        nc.sbuf_tensor([128, DB * NSH], BF16) as densbuf,
        nc.sbuf_tensor([128, NSH], F32) as dpcs,
        nc.psum_tensor([128, PSB * NSH], F32) as work,
        nc.psum_tensor([128, NSH], F32) as dpc_ps,
        nc.semaphore("dma_sem") as dma_sem,
        nc.semaphore("mm_sem") as mm_sem,      # inc per MM_main done
        nc.semaphore("exp_sem") as exp_sem,    # inc per exp done
        nc.semaphore("q_sem") as q_sem,        # inc per MM_q done
        nc.semaphore("dve_sem") as dve_sem,
        nc.Block() as block,
    ):
        AT = mega[:, 8192:8192 + NSH]
        INIT_R = bpk[0:4, 2 * 8192:2 * 8192 + NSH]

        @block.gpsimd
        def _(g):
            g.dma_start(out=mega[:], in_=d_mega[:]).then_inc(dma_sem, 16)
            g.dma_start(out=bpk[:], in_=d_bpk[:]).then_inc(dma_sem, 16)
            g.wait_ge(dve_sem, 1)
            g.dma_start(out=d_dpc[:], in_=dpcs[:]).then_inc(dma_sem, 16)

        @block.tensor
        def _(t):
            t.wait_ge(dma_sem, 32)
            for k in range(NT):
                w = work[:, (k % PSB) * NSH:(k % PSB + 1) * NSH]
                if k >= PSB:
                    t.wait_ge(exp_sem, k - PSB + 1)
                t.matmul(w, bpk[0:4, 8192 + 128 * k:8192 + 128 * (k + 1)],
                         INIT_R, start=True, stop=False)
                t.matmul(w, mega[:, 128 * k:128 * (k + 1)], AT,
                         start=False, stop=True).then_inc(mm_sem, 1)
                # Q-sum for previous tile (keeps PE busy while ACT works)
                if k >= 1:
                    j = k - 1
                    t.wait_ge(exp_sem, j + 1)
                    t.matmul(dpc_ps[:], bpk[:, 128 * j:128 * (j + 1)],
                             densbuf[:, (j % DB) * NSH:(j % DB + 1) * NSH],
                             start=(j == 0), stop=False).then_inc(q_sem, 1)
            j = NT - 1
            t.wait_ge(exp_sem, j + 1)
            t.matmul(dpc_ps[:], bpk[:, 128 * j:128 * (j + 1)],
                     densbuf[:, (j % DB) * NSH:(j % DB + 1) * NSH],
                     start=False, stop=True).then_inc(q_sem, 1)

        @block.scalar
        def _(s):
            for k in range(NT):
                s.wait_ge(mm_sem, k + 1)
                if k >= DB:
                    s.wait_ge(q_sem, k - DB + 1)
                s.activation(densbuf[:, (k % DB) * NSH:(k % DB + 1) * NSH],
                             work[:, (k % PSB) * NSH:(k % PSB + 1) * NSH],
                             AF.Exp).then_inc(exp_sem, 1)

        @block.vector
        def _(v):
            v.wait_ge(q_sem, NT)
            v.tensor_copy(dpcs[:], dpc_ps[:]).then_inc(dve_sem, 1)

    return nc


def _prep(a, b, var):
    c = -0.5 / var
    bf = b.reshape(M * Q, D).astype(np.float32)
    BT = np.ascontiguousarray(bf.T)                                  # [128, 8192]
    AT2 = (a.T.astype(np.float32) * np.float32(-2.0 * c))            # [128, 4096]
    a2 = (a.astype(np.float64) ** 2).sum(1)
    b2 = (bf.astype(np.float64) ** 2).sum(1)
    ca2 = (c * a2).astype(np.float32)                                # [4096]
    ca2_hi = ca2.astype(ml_dtypes.bfloat16).astype(np.float32)
    ca2_lo = (ca2 - ca2_hi).astype(np.float32)
    bias = (c * b2 + S_SHIFT).astype(np.float32)                     # [8192]
    bias_hi = bias.astype(ml_dtypes.bfloat16).astype(np.float32)
    bias_lo = (bias - bias_hi).astype(np.float32)

    # bf16 pack cols: [0:8192 QO blockones][8192:16384 init lhsT][16384: init rhs]
    bpk = np.zeros((128, 2 * 8192 + NSH), dtype=ml_dtypes.bfloat16)
    for k in range(NT):
        bpk[0:64, 128 * k + 2 * k] = 1.0
        bpk[64:128, 128 * k + 2 * k + 1] = 1.0
    bpk[0, 8192:16384] = 1.0
    bpk[1, 8192:16384] = 1.0
    bpk[2, 8192:16384] = bias_hi.astype(ml_dtypes.bfloat16)
    bpk[3, 8192:16384] = bias_lo.astype(ml_dtypes.bfloat16)
    bpk[2, 16384:] = 1.0
    bpk[3, 16384:] = 1.0

    in_maps = []
    for core in range(NCORES):
        sl = slice(core * NSH, (core + 1) * NSH)
        mega = np.concatenate([BT, AT2[:, sl]], axis=1).astype(np.float32)
        bp = bpk.copy()
        bp[0, 16384:] = ca2_hi[sl].astype(ml_dtypes.bfloat16)
        bp[1, 16384:] = ca2_lo[sl].astype(ml_dtypes.bfloat16)
        in_maps.append({"mega": mega, "bpk": bp})
    return in_maps, c


def _run(a, b, var, trace=False):
    from concourse.bass_utils import run_bass_kernel_spmd
    key = "nc"
    if key not in _cache:
        _cache[key] = _build()
    nc = _cache[key]
    in_maps, c = _prep(a, b, var)
    res = run_bass_kernel_spmd(nc, in_maps, list(range(NCORES)), trace=trace)
    eps_scaled = np.float32(1e-10 * float(np.exp(np.float64(S_SHIFT))))
    out = np.empty((N, M), dtype=np.float32)
    for core in range(NCORES):
        dpc = res.results[core]["dpc"]                   # [128 m, 512 n]
        dpc_nm = dpc.T.astype(np.float32)                # [512 n, 128 m]
        r = dpc_nm.sum(axis=1, keepdims=True, dtype=np.float32)
        out[core * NSH:(core + 1) * NSH] = dpc_nm / (r + eps_scaled)
    return out, res


def kernel(a_embeddings, b_embeddings=None, b_embedding_sets=None,
           gaussian_variance=None, **kw):
    b = b_embedding_sets if b_embedding_sets is not None else b_embeddings
    a = np.asarray(a_embeddings, dtype=np.float32)
    b = np.asarray(b, dtype=np.float32)
    var = float(np.asarray(gaussian_variance).reshape(-1)[0])
    out, _ = _run(a, b, var)
    return out
